# revision 2
# baseline (speedup 1.0000x reference)
"""AmplitudeEncoder Trainium2 kernel.

Computes, for x [64, 784] f32:
    state = pad(x, [.., 1001]); state /= ||state||_2 (per row)
    out[b] = outer(state[b], state[b])  -> [64, 1001, 1001] f32

Pure data-parallel across 8 NeuronCores: batch dim sharded 8 samples/core.
Per core the kernel is output-DMA bound (~32 MB of HBM writes).
"""

import numpy as np

import concourse.bacc as bacc
import concourse.tile as tile
from concourse import mybir
from concourse.bass_utils import run_bass_kernel_spmd

N_CORES = 8
B = 64  # full batch
F = 784  # features per sample
D = 1001  # statevector dim (comb(14, 4))
P = 128  # SBUF partitions
NCHUNK = 8  # ceil(D / P)
DP = NCHUNK * P  # 1024, padded statevector length
BSH = B // N_CORES  # samples per core
TAIL = D - 7 * P  # 105 rows in the last chunk

F32 = mybir.dt.float32

_compiled_nc = None


def _build():
    nc = bacc.Bacc("TRN2", debug=False)
    x = nc.dram_tensor("x", [BSH, F], F32, kind="ExternalInput")
    out = nc.dram_tensor("out", [BSH, D, D], F32, kind="ExternalOutput")
    s_dram = nc.dram_tensor("s_dram", [BSH, DP], F32)

    with tile.TileContext(nc) as tc:
        with (
            tc.tile_pool(name="small", bufs=1) as small,
            tc.tile_pool(name="rows", bufs=BSH) as rows,
            tc.tile_pool(name="outs", bufs=3) as outs,
        ):
            # ---- stage 1: L2-normalize each sample into a padded statevector
            x_t = small.tile([BSH, F], F32)
            nc.sync.dma_start(x_t[:], x.ap())
            sq = small.tile([BSH, F], F32)
            ssq = small.tile([BSH, 1], F32)
            nc.scalar.activation(
                sq[:], x_t[:], mybir.ActivationFunctionType.Square, accum_out=ssq[:]
            )
            nrm = small.tile([BSH, 1], F32)
            nc.scalar.sqrt(nrm[:], ssq[:])
            inv = small.tile([BSH, 1], F32)
            nc.vector.reciprocal(inv[:], nrm[:])
            s_t = small.tile([BSH, DP], F32)
            nc.vector.memset(s_t[:], 0.0)
            nc.vector.tensor_scalar_mul(s_t[:, :F], x_t[:], inv[:])
            nc.sync.dma_start(s_dram.ap(), s_t[:])

            # ---- stage 2: reshape the statevectors via DRAM round-trip
            # col_t[p, b, c] = s[b, c*128 + p]  (per-partition scalars)
            col_t = small.tile([P, BSH, NCHUNK], F32)
            nc.sync.dma_start(col_t[:], s_dram.ap().rearrange("b (c p) -> p b c", p=P))
            # row_ts[b][p, j] = s[b, j]  (row broadcast across partitions)
            row_ts = []
            for b in range(BSH):
                row_t = rows.tile([P, DP], F32, tag="row")
                nc.sync.dma_start(row_t[:], s_dram.ap()[b : b + 1, :].to_broadcast((P, DP)))
                row_ts.append(row_t)

            # ---- stage 3: outer products, one sample at a time
            for b in range(BSH):
                o_t = outs.tile([P, NCHUNK, DP], F32, tag="out")
                nc.vector.tensor_mul(
                    o_t[:, :, :D],
                    row_ts[b][:, None, :D].to_broadcast((P, NCHUNK, D)),
                    col_t[:, b, :][:, :, None].to_broadcast((P, NCHUNK, D)),
                )
                # chunks 0..6: rows c*128+p, full 128-partition blocks
                nc.sync.dma_start(
                    out.ap()[b, : 7 * P, :].rearrange("(c p) j -> p c j", p=P),
                    o_t[:, :7, :D],
                )
                # chunk 7: rows 896..1000 (105 rows)
                nc.sync.dma_start(out.ap()[b, 7 * P : D, :], o_t[:TAIL, 7, :D])

    nc.compile()
    return nc


def _get_nc():
    global _compiled_nc
    if _compiled_nc is None:
        _compiled_nc = _build()
    return _compiled_nc


def run_sharded(x: np.ndarray, trace: bool = False):
    """Run the SPMD kernel; returns (full_output, BassKernelResults)."""
    x = np.ascontiguousarray(np.asarray(x, dtype=np.float32))
    assert x.shape == (B, F), x.shape
    nc = _get_nc()
    in_maps = [{"x": x[i * BSH : (i + 1) * BSH]} for i in range(N_CORES)]
    res = run_bass_kernel_spmd(nc, in_maps, core_ids=list(range(N_CORES)), trace=trace)
    out = np.concatenate([res.results[i]["out"] for i in range(N_CORES)], axis=0)
    return out, res


def kernel(x: np.ndarray) -> np.ndarray:
    out, _ = run_sharded(x)
    return out


# revision 11
# speedup vs baseline: 1.0158x; 1.0158x over previous
"""AmplitudeEncoder Trainium2 kernel.

Computes, for x [64, 784] f32:
    state = pad(x, [.., 1001]); state /= ||state||_2 (per row)
    out[b] = outer(state[b], state[b])  -> [64, 1001, 1001] f32

Pure data-parallel across 8 NeuronCores: batch dim sharded 8 samples/core.
Per core the kernel is output-DMA bound (~32 MB of HBM writes ~= 90us at
358 GB/s).

Per-core dataflow:
  stage 1 (tiny): load x [8,784]; sum-of-squares -> sqrt -> reciprocal;
      scale into padded statevector s_t [8, 1024] (sample per partition).
  stage 2 (PE): 8 transpose-matmuls give col layout psum_col[p, c, b] =
      s[b, c*128+p]; copied to SBUF. Per sample, 2 K=1 matmuls with a ones
      row broadcast s[b, :] across partitions into PSUM prow [128, 1024].
  stage 3: out_tile[p, c, j] = prow[p, j] * col[p, c, b]; chunks 0..4 on
      DVE (one 3D broadcast tensor_tensor), chunks 5..7 on ACT (per-chunk
      activation Copy with per-partition scale). Two DMAs per sample write
      out[b] (rows c*128+p), issue spread across sync/gpsimd/tensor.
"""

import numpy as np

import concourse.bacc as bacc
import concourse.tile as tile
from concourse import mybir
from concourse.bass_utils import run_bass_kernel_spmd

N_CORES = 8
B = 64  # full batch
F = 784  # features per sample
D = 1001  # statevector dim (comb(14, 4))
P = 128  # SBUF partitions
NCHUNK = 8  # ceil(D / P)
DP = NCHUNK * P  # 1024, padded statevector length
BSH = B // N_CORES  # samples per core
TAIL = D - 7 * P  # 105 rows in the last chunk
DVE_CHUNKS = 5  # chunks 0..4 on vector engine, 5..7 on scalar engine

F32 = mybir.dt.float32

_compiled_nc = None


def _consts() -> np.ndarray:
    """[8, 1032] f32: per-sample broadcast masks [8, 1024] ++ identity [8, 8].

    masks[:, b*P:(b+1)*P] is an [8, 128] selection matrix whose row b is
    all-ones: masks_b.T @ s_t broadcasts sample b's row across all 128
    output partitions (matmul base partition must be 0, so K=8 selection
    replaces a K=1 per-partition slice). The identity feeds PE transpose.
    """
    masks = np.zeros((BSH, BSH, P), dtype=np.float32)
    for b in range(BSH):
        masks[b, b, :] = 1.0
    ident = np.eye(BSH, dtype=np.float32)
    return np.concatenate([masks.reshape(BSH, BSH * P), ident], axis=1)


def _build():
    nc = bacc.Bacc("TRN2", debug=False)
    x = nc.dram_tensor("x", [BSH, F], F32, kind="ExternalInput")
    consts = nc.dram_tensor("consts", [BSH, BSH * P + BSH], F32, kind="ExternalInput")
    out = nc.dram_tensor("out", [BSH, D, D], F32, kind="ExternalOutput")

    with tile.TileContext(nc) as tc:
        with (
            tc.tile_pool(name="small", bufs=1) as small,
            tc.tile_pool(name="pcol", bufs=1, space="PSUM") as pcolp,
            tc.tile_pool(name="prow", bufs=3, space="PSUM") as prowp,
            tc.tile_pool(name="outs", bufs=4) as outs,
        ):
            # ---- constants (masks ++ identity, see _consts)
            consts_t = small.tile([BSH, BSH * P + BSH], F32)
            nc.gpsimd.dma_start(consts_t[:], consts.ap())
            masks = consts_t[:, : BSH * P]
            ident = consts_t[:, BSH * P :]

            # ---- stage 1: L2-normalize each sample into padded statevector
            x_t = small.tile([BSH, F], F32)
            nc.gpsimd.dma_start(x_t[:], x.ap())
            sq = small.tile([BSH, F], F32)
            ssq = small.tile([BSH, 1], F32)
            nc.scalar.activation(
                sq[:], x_t[:], mybir.ActivationFunctionType.Square, accum_out=ssq[:]
            )
            nrm = small.tile([BSH, 1], F32)
            nc.scalar.sqrt(nrm[:], ssq[:])
            inv = small.tile([BSH, 1], F32)
            nc.vector.reciprocal(inv[:], nrm[:])
            s_t = small.tile([BSH, DP], F32)
            nc.vector.memset(s_t[:], 0.0)
            nc.vector.tensor_scalar_mul(s_t[:, :F], x_t[:], inv[:])

            # ---- stage 2a: column layout via PE transpose
            # psum_col[p, c, b] = s_t[b, c*128+p]
            psum_col = pcolp.tile([P, NCHUNK, BSH], F32, tag="pcol")
            for c in range(NCHUNK):
                nc.tensor.transpose(
                    psum_col[:, c, :], s_t[:, c * P : (c + 1) * P], ident
                )
            col_sb = small.tile([P, NCHUNK, BSH], F32)
            nc.vector.tensor_copy(col_sb[:], psum_col[:])

            # ---- stages 2b/3 per sample
            for b in range(BSH):
                # row broadcast into PSUM: prow[p, j] = s_t[b, j]
                prow = prowp.tile([P, DP], F32, tag="prow")
                nc.tensor.matmul(
                    prow[:, :512],
                    lhsT=masks[:, b * P : (b + 1) * P],
                    rhs=s_t[:, :512],
                    start=True,
                    stop=True,
                )
                nc.tensor.matmul(
                    prow[:, 512:],
                    lhsT=masks[:, b * P : (b + 1) * P],
                    rhs=s_t[:, 512:],
                    start=True,
                    stop=True,
                )

                o_t = outs.tile([P, NCHUNK, DP], F32, tag="out")
                nc.vector.tensor_tensor(
                    o_t[:, :DVE_CHUNKS, :D],
                    prow[:, None, :D].to_broadcast((P, DVE_CHUNKS, D)),
                    col_sb[:, :DVE_CHUNKS, b][:, :, None].to_broadcast(
                        (P, DVE_CHUNKS, D)
                    ),
                    mybir.AluOpType.mult,
                )
                for c in range(DVE_CHUNKS, NCHUNK):
                    nc.scalar.mul(
                        o_t[:, c, :D], prow[:, :D], col_sb[:, c, b : b + 1]
                    )

                big_eng = [nc.sync, nc.gpsimd][b % 2]
                tail_eng = [nc.gpsimd, nc.sync][b % 2]
                big_eng.dma_start(
                    out.ap()[b, : 7 * P, :].rearrange("(c p) j -> p c j", p=P),
                    o_t[:, :7, :D],
                )
                tail_eng.dma_start(out.ap()[b, 7 * P : D, :], o_t[:TAIL, 7, :D])

    nc.compile()
    return nc


def _get_nc():
    global _compiled_nc
    if _compiled_nc is None:
        _compiled_nc = _build()
    return _compiled_nc


def run_sharded(x: np.ndarray, trace: bool = False):
    """Run the SPMD kernel; returns (full_output, BassKernelResults)."""
    x = np.ascontiguousarray(np.asarray(x, dtype=np.float32))
    assert x.shape == (B, F), x.shape
    nc = _get_nc()
    consts = _consts()
    in_maps = [
        {"x": x[i * BSH : (i + 1) * BSH], "consts": consts} for i in range(N_CORES)
    ]
    res = run_bass_kernel_spmd(nc, in_maps, core_ids=list(range(N_CORES)), trace=trace)
    out = np.concatenate([res.results[i]["out"] for i in range(N_CORES)], axis=0)
    return out, res


def kernel(x: np.ndarray) -> np.ndarray:
    out, _ = run_sharded(x)
    return out
